# revision 1
# baseline (speedup 1.0000x reference)
"""Trainium2 Bass kernel for nn_LocalFeatureEncoder.

Computes, for B=8 batches on 8 NeuronCores (batch b -> core b):
    g      = concat(shape_code, structure_code, pose_code)      # (B, 128)
    local  = einsum('kfz,bz->bkf', W, g) + bias                 # (B, 24, 64)
    out    = einsum('btk,bkf->btf', lbs_weights, local)         # (B, 32768, 64)

Per-core device program:
  Stage 1: broadcast g across partitions with a rank-1 PE matmul; compute
    local in "column" layout (kf across partitions) with 12 DVE
    tensor_tensor_reduce ops; PE-transpose + tiny DRAM roundtrip to lay
    local+bias out as a block-diagonal [96, 256] matmul rhs.
  Stage 2: lbs (32768,24) loaded as flat [128, 6144] (partition p holds
    t-rows p*256..p*256+255). Per group of 4 t-rows/partition: PE transpose
    [128,96] -> [96,128], copy PSUM->SBUF, one matmul
    [96,128]^T @ [96,256] -> [128,256] (= out rows t=p*256+g*4+j), copy to
    staging, and one 512KB DMA out per 4 groups.
"""

import os
from contextlib import ExitStack

import numpy as np

import concourse.bass as bass
import concourse.bacc as bacc
import concourse.tile as tile
from concourse import mybir
from concourse import bass_utils

B, T, K, Z, F = 8, 32768, 24, 128, 64
P = 128                 # SBUF partitions
TPB = T // P            # 256 t-rows per partition
CHUNK = 4               # t-rows per group
GCOLS = CHUNK * K       # 96 transpose-input columns per group
NF = CHUNK * F          # 256 matmul output columns per group
NGROUPS = TPB // CHUNK  # 64
NBATCH = 4              # groups per output staging batch
NQ = NGROUPS // NBATCH  # 16 output DMAs
NWTILE = (K * F) // P   # 12 W tiles of [128, 128]
IN_CHUNKS = 8           # lbs load split

_built = {}


def _build(use_f32r: bool):
    key = use_f32r
    if key in _built:
        return _built[key]

    f32 = mybir.dt.float32
    f32r = mybir.dt.float32r
    mm_dt = f32r if use_f32r else f32
    nc = bacc.Bacc("TRN2", target_bir_lowering=False, debug=False)

    lbs_d = nc.dram_tensor("lbs", (P, TPB * K), f32, kind="ExternalInput")
    g_d = nc.dram_tensor("g", (1, Z), f32, kind="ExternalInput")
    w_d = nc.dram_tensor("w", (NWTILE, P, Z), f32, kind="ExternalInput")
    biasc_d = nc.dram_tensor("biasc", (P, NWTILE), f32, kind="ExternalInput")
    ident_d = nc.dram_tensor("ident", (P, P), f32, kind="ExternalInput")
    bdzero_d = nc.dram_tensor("bdzero", (GCOLS, NF), mm_dt, kind="ExternalInput")
    out_d = nc.dram_tensor("out", (P, TPB * F), f32, kind="ExternalOutput")

    with tile.TileContext(nc) as tc, ExitStack() as ctx:
        const = ctx.enter_context(tc.tile_pool(name="const", bufs=1))
        big = ctx.enter_context(tc.tile_pool(name="big", bufs=1))
        dram = ctx.enter_context(
            tc.tile_pool(name="dram", bufs=1, space=bass.MemorySpace.DRAM)
        )
        ps1 = ctx.enter_context(
            tc.tile_pool(name="ps1", bufs=1, space=bass.MemorySpace.PSUM)
        )
        psT = ctx.enter_context(
            tc.tile_pool(name="psT", bufs=3, space=bass.MemorySpace.PSUM)
        )
        psO = ctx.enter_context(
            tc.tile_pool(name="psO", bufs=3, space=bass.MemorySpace.PSUM)
        )
        sbT_pool = ctx.enter_context(tc.tile_pool(name="sbT_pool", bufs=4))
        stag_pool = ctx.enter_context(tc.tile_pool(name="stag_pool", bufs=3))

        # ---- constant loads ----
        ident = const.tile([P, P], f32)
        nc.sync.dma_start(ident[:], ident_d.ap())

        lbs_sb = big.tile([P, TPB * K], f32)
        cw = (TPB * K) // IN_CHUNKS
        for c in range(IN_CHUNKS):
            nc.sync.dma_start(
                lbs_sb[:, c * cw:(c + 1) * cw], lbs_d.ap()[:, c * cw:(c + 1) * cw]
            )

        w_sb = const.tile([P, NWTILE * Z], f32)
        # w_d is (n, p, z); SBUF wants [p, (n z)]
        nc.sync.dma_start(
            w_sb[:].rearrange("p (n z) -> p n z", n=NWTILE),
            w_d.ap().rearrange("n p z -> p n z"),
        )

        g_sb = const.tile([1, Z], f32)
        nc.sync.dma_start(g_sb[:], g_d.ap())

        biascol = const.tile([P, NWTILE], f32)
        nc.sync.dma_start(biascol[:], biasc_d.ap())

        # ---- stage 1: local = W @ g + bias, laid out block-diagonally ----
        ones = const.tile([1, P], f32)
        nc.vector.memset(ones[:], 1.0)
        gb_ps = ps1.tile([P, Z], f32, tag="s1")
        nc.tensor.matmul(gb_ps[:], ones[:], g_sb[:], start=True, stop=True)
        g_bc = const.tile([P, Z], f32)
        nc.scalar.copy(g_bc[:], gb_ps[:])

        localcol = const.tile([P, NWTILE], f32)
        prodw = const.tile([P, NWTILE * Z], f32)
        for n in range(NWTILE):
            nc.vector.tensor_mul(
                prodw[:, n * Z:(n + 1) * Z], w_sb[:, n * Z:(n + 1) * Z], g_bc[:]
            )
        nc.vector.reduce_sum(
            localcol[:],
            prodw[:].rearrange("p (n z) -> p n z", n=NWTILE),
            axis=mybir.AxisListType.X,
        )
        # add bias while in column layout
        nc.vector.tensor_add(localcol[:], localcol[:], biascol[:])

        # transpose [128, 12] -> [12, 128] and roundtrip via DRAM so the
        # (k,f) axis lands on partitions
        lT_ps = ps1.tile([NWTILE, P], f32, tag="s1")
        nc.tensor.transpose(lT_ps[:], localcol[:], ident[:])
        lT_sb = const.tile([NWTILE, P], mm_dt)
        nc.vector.tensor_copy(lT_sb[:], lT_ps[:])
        scratch = dram.tile([NWTILE, P], mm_dt)
        nc.sync.dma_start(scratch[:], lT_sb[:])

        bd = const.tile([GCOLS, NF], mm_dt)
        nc.sync.dma_start(bd[:], bdzero_d.ap())
        for j in range(CHUNK):
            nc.sync.dma_start(
                bd[j * K:(j + 1) * K, j * F:(j + 1) * F],
                scratch[:].rearrange("n (h f) -> (n h) f", h=2),
            )

        # ---- stage 2: main loop over 64 groups ----
        for q in range(NQ):
            stag = stag_pool.tile([P, NBATCH * NF], f32)
            for j in range(NBATCH):
                gi = q * NBATCH + j
                tp = psT.tile([GCOLS, P], f32)
                nc.tensor.transpose(
                    tp[:], lbs_sb[:, gi * GCOLS:(gi + 1) * GCOLS], ident[:]
                )
                sbT = sbT_pool.tile([GCOLS, P], mm_dt)
                if gi % 2 == 0:
                    nc.vector.tensor_copy(sbT[:], tp[:])
                else:
                    nc.scalar.copy(sbT[:], tp[:])
                ops = psO.tile([P, NF], f32)
                nc.tensor.matmul(ops[:], sbT[:], bd[:], start=True, stop=True)
                if gi % 2 == 0:
                    nc.scalar.copy(stag[:, j * NF:(j + 1) * NF], ops[:])
                else:
                    nc.vector.tensor_copy(stag[:, j * NF:(j + 1) * NF], ops[:])
            nc.sync.dma_start(
                out_d.ap()[:, q * NBATCH * NF:(q + 1) * NBATCH * NF], stag[:]
            )

    nc.compile()
    _built[key] = nc
    return nc


def make_in_maps(inputs):
    g_full = np.concatenate(
        [inputs["shape_code"], inputs["structure_code"], inputs["pose_code"]],
        axis=-1,
    ).astype(np.float32)  # (8, 128)
    w_r = np.ascontiguousarray(
        inputs["W"].astype(np.float32).reshape(NWTILE, P, Z)
    )
    # bias in "column" layout: biascol[p, n] = bias.flat[n*128 + p]
    biasc = np.ascontiguousarray(
        inputs["bias"].astype(np.float32).reshape(NWTILE, P).T
    )
    ident = np.eye(P, dtype=np.float32)
    lbs = inputs["lbs_weights"].astype(np.float32)
    in_maps = []
    for b in range(B):
        in_maps.append(
            {
                "lbs": np.ascontiguousarray(lbs[b].reshape(P, TPB * K)),
                "g": g_full[b:b + 1],
                "w": w_r,
                "biasc": biasc,
                "ident": ident,
                "bdzero": np.zeros((GCOLS, NF), dtype=np.float32),
            }
        )
    return in_maps


LAST_RESULT = None


def kernel(**inputs) -> np.ndarray:
    global LAST_RESULT
    use_f32r = os.environ.get("LFE_F32R", "1") == "1"
    nc = _build(use_f32r)
    in_maps = make_in_maps(inputs)
    res = bass_utils.run_bass_kernel_spmd(
        nc,
        in_maps,
        core_ids=list(range(B)),
        trace=os.environ.get("LFE_TRACE", "0") == "1",
    )
    LAST_RESULT = res
    out = np.stack(
        [res.results[b]["out"].reshape(T, F) for b in range(B)], axis=0
    )
    return out


if __name__ == "__main__":
    rng = np.random.default_rng(0)
    inputs = {
        "shape_code": rng.standard_normal((B, 64), dtype=np.float32),
        "structure_code": rng.standard_normal((B, 32), dtype=np.float32),
        "pose_code": rng.standard_normal((B, 32), dtype=np.float32),
        "lbs_weights": rng.random((B, T, K), dtype=np.float32),
        "W": rng.standard_normal((K, F, Z), dtype=np.float32),
        "bias": rng.standard_normal((K, F), dtype=np.float32),
    }
    out = kernel(**inputs)
    g = np.concatenate(
        [inputs["shape_code"], inputs["structure_code"], inputs["pose_code"]], -1
    )
    local = np.einsum("kfz,bz->bkf", inputs["W"], g) + inputs["bias"][None]
    ref = np.einsum("btk,bkf->btf", inputs["lbs_weights"], local)
    err = np.abs(out - ref).max() / np.abs(ref).max()
    print("rel err:", err)



# revision 3
# speedup vs baseline: 1.7159x; 1.7159x over previous
"""Trainium2 Bass kernel for nn_LocalFeatureEncoder.

Computes, for B=8 batches on 8 NeuronCores (batch b -> core b):
    g      = concat(shape_code, structure_code, pose_code)      # (B, 128)
    local  = einsum('kfz,bz->bkf', W, g) + bias                 # (B, 24, 64)
    out    = einsum('btk,bkf->btf', lbs_weights, local)         # (B, 32768, 64)

Design (v2 — memory-roofline oriented):
  * Host pre-shuffles lbs into bf16 [128, 8192]: partition (j,k') holds
    lbs[t=(tau*128+i)*4+j, k'] at column tau*128+i (k' zero-padded 24->32).
    This puts the contraction axis on partitions, so the main loop needs
    NO PE transposes and no transpose copies.
  * local is computed on device: flat[i, k*64+f] = sum_z g[z] W[k,f,z] via
    3 matmuls (lhsT = g replicated), then scattered into a block-diagonal
    bf16 rhs bd[128, 256] with 4 tiny SBUF->SBUF DMAs + bias add.
  * Main loop: 64 matmuls [128,128]^T @ [128,256] -> PSUM f32, drained in
    [128,512] pairs by alternating DVE/ACT cast-copies to bf16 staging,
    stored with 8 x 512KB DMAs.
  * bf16 end-to-end (inputs host-cast, output host-upcast): per-core HBM
    traffic ~6.8 MB vs 11.5 MB for f32.
  * Input lbs chunks stream on the sync HWDGE ring; stage-1 constants ride
    the scalar ring so they are not FIFO-blocked behind the bulk load.
"""

import os

import numpy as np
import ml_dtypes

import concourse.bass as bass
import concourse.bacc as bacc
import concourse.tile as tile
from concourse import mybir
from concourse import bass_utils
from contextlib import ExitStack

B, T, K, Z, F = 8, 32768, 24, 128, 64
P = 128
JG = 4                  # t-offsets per partition block
KP = 32                 # padded joints per block (24 used)
TPT = 128               # t-groups (columns of stationary) per tile
NTILES = T // (JG * TPT)       # 64
NF = JG * F                    # 256 bd/matmul output cols
NCOLS = NTILES * TPT           # 8192 lbs cols
IN_CHUNKS = 4
CHUNK_COLS = NCOLS // IN_CHUNKS    # 2048
TILES_PER_CHUNK = NTILES // IN_CHUNKS  # 16
PAIR = 2                # matmul tiles per PSUM bank drain
SBATCH = 8              # tiles per output store (8*256 cols bf16 = 4KB/part)
NSTORES = NTILES // SBATCH     # 8
KF = K * F              # 1536

_built = {}


def _build():
    if "nc" in _built:
        return _built["nc"]

    f32 = mybir.dt.float32
    bf16 = mybir.dt.bfloat16
    nc = bacc.Bacc("TRN2", target_bir_lowering=False, debug=False)

    lbs_d = nc.dram_tensor("lbs", (P, NCOLS), bf16, kind="ExternalInput")
    grep_d = nc.dram_tensor("grep", (P, P), bf16, kind="ExternalInput")
    wt_d = nc.dram_tensor("wt", (P, KF), bf16, kind="ExternalInput")
    biasbd_d = nc.dram_tensor("biasbd", (P, NF), bf16, kind="ExternalInput")
    out_d = nc.dram_tensor("out", (P, NTILES * NF), bf16, kind="ExternalOutput")

    with tile.TileContext(nc) as tc, ExitStack() as ctx:
        const = ctx.enter_context(tc.tile_pool(name="const", bufs=1))
        lbs_pool = ctx.enter_context(tc.tile_pool(name="lbs_pool", bufs=IN_CHUNKS))
        psS = ctx.enter_context(
            tc.tile_pool(name="psS", bufs=2, space=bass.MemorySpace.PSUM)
        )
        psO = ctx.enter_context(
            tc.tile_pool(name="psO", bufs=3, space=bass.MemorySpace.PSUM)
        )
        stag_pool = ctx.enter_context(tc.tile_pool(name="stag_pool", bufs=2))

        # ---- stage-1 constants FIRST on the sync ring: they are small and
        # complete in ~1.5us; lbs queues behind them FIFO ----
        grep_sb = const.tile([P, P], bf16)
        nc.sync.dma_start(grep_sb[:], grep_d.ap())
        wt_sb = const.tile([P, KF], bf16)
        nc.sync.dma_start(wt_sb[:], wt_d.ap())
        biasbd_sb = const.tile([P, NF], bf16)
        nc.sync.dma_start(biasbd_sb[:], biasbd_d.ap())

        # ---- bulk lbs stream on the sync ring ----
        lbs_sb = []
        for c in range(IN_CHUNKS):
            t = lbs_pool.tile([P, CHUNK_COLS], bf16)
            nc.sync.dma_start(
                t[:], lbs_d.ap()[:, c * CHUNK_COLS:(c + 1) * CHUNK_COLS]
            )
            lbs_sb.append(t)

        # ---- stage 1: flat[i, k*64+f] = sum_z g[z] W[k,f,z]  (rows identical) ----
        bdt = const.tile([P, NF], bf16)
        nc.vector.memset(bdt[:], 0.0)

        flat_sb = const.tile([P, KF], bf16)
        for n in range(3):
            fp = psS.tile([P, 512], f32, tag="s1")
            nc.tensor.matmul(
                fp[:], grep_sb[:], wt_sb[:, n * 512:(n + 1) * 512],
                start=True, stop=True,
            )
            if n == 1:
                nc.scalar.copy(flat_sb[:, n * 512:(n + 1) * 512], fp[:])
            else:
                nc.vector.tensor_copy(flat_sb[:, n * 512:(n + 1) * 512], fp[:])

        # scatter row 0 of flat into the diagonal blocks of bdt (SBUF->SBUF);
        # split across the two HWDGE issuing engines to halve issue latency
        for j in range(JG):
            eng = nc.scalar if j % 2 else nc.sync
            eng.dma_start(
                bdt[j * KP:j * KP + K, j * F:(j + 1) * F], flat_sb[0:1, :]
            )
        bd = const.tile([P, NF], bf16)
        nc.vector.tensor_add(bd[:], bdt[:], biasbd_sb[:])

        # ---- main loop: 64 matmuls, drained in 2-bank quads, 8 store batches ----
        QUAD = 4
        for s in range(NSTORES):
            stag = stag_pool.tile([P, SBATCH * NF], bf16)
            for q in range(SBATCH // QUAD):
                op = psO.tile([P, QUAD * NF], f32)
                for h in range(QUAD):
                    ti = s * SBATCH + q * QUAD + h
                    c = ti // TILES_PER_CHUNK
                    col = (ti % TILES_PER_CHUNK) * TPT
                    nc.tensor.matmul(
                        op[:, h * NF:(h + 1) * NF],
                        lbs_sb[c][:, col:col + TPT],
                        bd[:],
                        start=True, stop=True,
                    )
                dst = stag[:, q * QUAD * NF:(q + 1) * QUAD * NF]
                if (s * 2 + q) % 2 == 0:
                    nc.vector.tensor_copy(dst, op[:])
                else:
                    nc.scalar.copy(dst, op[:])
            seng = nc.sync if s % 2 == 0 else nc.scalar
            seng.dma_start(
                out_d.ap()[:, s * SBATCH * NF:(s + 1) * SBATCH * NF], stag[:]
            )

    nc.compile()
    _built["nc"] = nc
    return nc


def make_in_maps(inputs):
    bf16 = ml_dtypes.bfloat16
    g_full = np.concatenate(
        [inputs["shape_code"], inputs["structure_code"], inputs["pose_code"]],
        axis=-1,
    ).astype(np.float32)  # (8, 128)
    # wt[z, k*64+f] = W[k, f, z]
    wt = np.ascontiguousarray(
        inputs["W"].astype(np.float32).transpose(2, 0, 1).reshape(P, KF)
    ).astype(bf16)
    # biasbd: block-diagonal bias, k' padded to 32
    bias = inputs["bias"].astype(np.float32)
    biasbd = np.zeros((JG, KP, NF), dtype=np.float32)
    for j in range(JG):
        biasbd[j, :K, j * F:(j + 1) * F] = bias
    biasbd = biasbd.reshape(P, NF).astype(bf16)

    lbs = inputs["lbs_weights"].astype(np.float32)
    in_maps = []
    for b in range(B):
        # lbs4[j*32+k', tau*128+i] = lbs[b, (tau*128+i)*4+j, k']
        lb = lbs[b].reshape(NCOLS, JG, K).transpose(1, 2, 0)  # (JG, K, 8192)
        lbs4 = np.zeros((JG, KP, NCOLS), dtype=np.float32)
        lbs4[:, :K, :] = lb
        grep = np.ascontiguousarray(
            np.broadcast_to(g_full[b][:, None], (P, P))
        ).astype(bf16)
        in_maps.append(
            {
                "lbs": np.ascontiguousarray(lbs4.reshape(P, NCOLS)).astype(bf16),
                "grep": grep,
                "wt": wt,
                "biasbd": biasbd,
            }
        )
    return in_maps


LAST_RESULT = None


def kernel(**inputs) -> np.ndarray:
    global LAST_RESULT
    nc = _build()
    in_maps = make_in_maps(inputs)
    res = bass_utils.run_bass_kernel_spmd(
        nc,
        in_maps,
        core_ids=list(range(B)),
        trace=os.environ.get("LFE_TRACE", "0") == "1",
    )
    LAST_RESULT = res
    outs = []
    for b in range(B):
        o = np.asarray(res.results[b]["out"]).astype(np.float32)
        # out_d[p, tau*256 + j*64 + f] = out[(tau*128+p)*4+j, f]
        o = o.reshape(P, NTILES, JG, F).transpose(1, 0, 2, 3).reshape(T, F)
        outs.append(o)
    return np.stack(outs, axis=0)


if __name__ == "__main__":
    rng = np.random.default_rng(0)
    inputs = {
        "shape_code": rng.standard_normal((B, 64), dtype=np.float32),
        "structure_code": rng.standard_normal((B, 32), dtype=np.float32),
        "pose_code": rng.standard_normal((B, 32), dtype=np.float32),
        "lbs_weights": rng.random((B, T, K), dtype=np.float32),
        "W": rng.standard_normal((K, F, Z), dtype=np.float32),
        "bias": rng.standard_normal((K, F), dtype=np.float32),
    }
    out = kernel(**inputs)
    g = np.concatenate(
        [inputs["shape_code"], inputs["structure_code"], inputs["pose_code"]], -1
    )
    local = np.einsum("kfz,bz->bkf", inputs["W"], g) + inputs["bias"][None]
    ref = np.einsum("btk,bkf->btf", inputs["lbs_weights"], local)
    err = np.abs(out - ref).max() / np.abs(ref).max()
    print("rel err:", err)


# revision 7
# speedup vs baseline: 1.9087x; 1.1123x over previous
"""Trainium2 Bass kernel for nn_LocalFeatureEncoder.

Computes, for B=8 batches on 8 NeuronCores (batch b -> core b):
    g      = concat(shape_code, structure_code, pose_code)      # (B, 128)
    local  = einsum('kfz,bz->bkf', W, g) + bias                 # (B, 24, 64)
    out    = einsum('btk,bkf->btf', lbs_weights, local)         # (B, 32768, 64)

Design (v2 — memory-roofline oriented):
  * Host pre-shuffles lbs into bf16 [128, 8192]: partition (j,k') holds
    lbs[t=(tau*128+i)*4+j, k'] at column tau*128+i (k' zero-padded 24->32).
    This puts the contraction axis on partitions, so the main loop needs
    NO PE transposes and no transpose copies.
  * local is computed on device: flat[i, k*64+f] = sum_z g[z] W[k,f,z] via
    3 matmuls (lhsT = g replicated), then scattered into a block-diagonal
    bf16 rhs bd[128, 256] with 4 tiny SBUF->SBUF DMAs + bias add.
  * Main loop: 64 matmuls [128,128]^T @ [128,256] -> PSUM f32, drained in
    [128,512] pairs by alternating DVE/ACT cast-copies to bf16 staging,
    stored with 8 x 512KB DMAs.
  * bf16 end-to-end (inputs host-cast, output host-upcast): per-core HBM
    traffic ~6.8 MB vs 11.5 MB for f32.
  * Input lbs chunks stream on the sync HWDGE ring; stage-1 constants ride
    the scalar ring so they are not FIFO-blocked behind the bulk load.
"""

import os

import numpy as np
import ml_dtypes

import concourse.bass as bass
import concourse.bacc as bacc
import concourse.tile as tile
from concourse import mybir
from concourse import bass_utils
from contextlib import ExitStack

B, T, K, Z, F = 8, 32768, 24, 128, 64
P = 128
JG = 4                  # t-offsets per partition block
KP = 32                 # padded joints per block (24 used)
TPT = 128               # t-groups (columns of stationary) per tile
NTILES = T // (JG * TPT)       # 64
NF = JG * F                    # 256 bd/matmul output cols
NCOLS = NTILES * TPT           # 8192 lbs cols
IN_CHUNKS = 4
CHUNK_COLS = NCOLS // IN_CHUNKS    # 2048
TILES_PER_CHUNK = NTILES // IN_CHUNKS  # 16
PAIR = 2                # matmul tiles per PSUM bank drain
SBATCH = 8              # tiles per output store (8*256 cols bf16 = 4KB/part)
NSTORES = NTILES // SBATCH     # 8
KF = K * F              # 1536

_built = {}


def _build(hostbd=False):
    key = ("hostbd" if hostbd else "nc")
    if key in _built:
        return _built[key]

    f32 = mybir.dt.float32
    bf16 = mybir.dt.bfloat16
    nc = bacc.Bacc("TRN2", target_bir_lowering=False, debug=False)

    lbs_d = nc.dram_tensor("lbs", (P, NCOLS), bf16, kind="ExternalInput")
    if hostbd:
        bd_d = nc.dram_tensor("bd", (P, NF), bf16, kind="ExternalInput")
    else:
        grep_d = nc.dram_tensor("grep", (P, P), bf16, kind="ExternalInput")
        wt_d = nc.dram_tensor("wt", (P, KF), bf16, kind="ExternalInput")
        biasbd_d = nc.dram_tensor("biasbd", (P, NF), bf16, kind="ExternalInput")
    out_d = nc.dram_tensor("out", (P, NTILES * NF), bf16, kind="ExternalOutput")

    with tile.TileContext(nc) as tc, ExitStack() as ctx:
        const = ctx.enter_context(tc.tile_pool(name="const", bufs=1))
        lbs_pool = ctx.enter_context(tc.tile_pool(name="lbs_pool", bufs=IN_CHUNKS))
        psS = ctx.enter_context(
            tc.tile_pool(name="psS", bufs=2, space=bass.MemorySpace.PSUM)
        )
        psO = ctx.enter_context(
            tc.tile_pool(name="psO", bufs=3, space=bass.MemorySpace.PSUM)
        )
        stag_pool = ctx.enter_context(tc.tile_pool(name="stag_pool", bufs=2))

        # ---- stage-1 constants FIRST on the sync ring: they are small and
        # complete in ~1.5us; lbs queues behind them FIFO ----
        if hostbd:
            bd = const.tile([P, NF], bf16)
            nc.sync.dma_start(bd[:], bd_d.ap())
        else:
            wt_sb = const.tile([P, KF], bf16)
            nc.sync.dma_start(wt_sb[:], wt_d.ap())
            grep_sb = const.tile([P, P], bf16)
            nc.sync.dma_start(grep_sb[:], grep_d.ap())
            biasbd_sb = const.tile([P, NF], bf16)
            nc.sync.dma_start(biasbd_sb[:], biasbd_d.ap())

        # ---- bulk lbs stream on the sync ring ----
        lbs_sb = []
        for c in range(IN_CHUNKS):
            t = lbs_pool.tile([P, CHUNK_COLS], bf16)
            nc.sync.dma_start(
                t[:], lbs_d.ap()[:, c * CHUNK_COLS:(c + 1) * CHUNK_COLS]
            )
            lbs_sb.append(t)

        if not hostbd:
            # ---- stage 1: flat[i, k*64+f] = sum_z g[z] W[k,f,z] (rows identical) ----
            bdt = const.tile([P, NF], bf16)
            nc.vector.memset(bdt[:], 0.0)

            flat_sb = const.tile([P, KF], bf16)
            for n in range(3):
                fp = psS.tile([P, 512], f32, tag="s1")
                nc.tensor.matmul(
                    fp[:], grep_sb[:], wt_sb[:, n * 512:(n + 1) * 512],
                    start=True, stop=True,
                )
                if n == 1:
                    nc.scalar.copy(flat_sb[:, n * 512:(n + 1) * 512], fp[:])
                else:
                    nc.vector.tensor_copy(flat_sb[:, n * 512:(n + 1) * 512], fp[:])

            # scatter row 0 of flat into the diagonal blocks of bdt; the
            # scalar ring (Q10) is empty and ACT is otherwise idle here
            for j in range(JG):
                nc.scalar.dma_start(
                    bdt[j * KP:j * KP + K, j * F:(j + 1) * F], flat_sb[0:1, :]
                )
            bd = const.tile([P, NF], bf16)
            nc.vector.tensor_add(bd[:], bdt[:], biasbd_sb[:])

        # ---- main loop: 64 matmuls, drained in 2-bank quads, 8 store batches ----
        QUAD = 4
        for s in range(NSTORES):
            stag = stag_pool.tile([P, SBATCH * NF], bf16)
            for q in range(SBATCH // QUAD):
                op = psO.tile([P, QUAD * NF], f32)
                for h in range(QUAD):
                    ti = s * SBATCH + q * QUAD + h
                    c = ti // TILES_PER_CHUNK
                    col = (ti % TILES_PER_CHUNK) * TPT
                    nc.tensor.matmul(
                        op[:, h * NF:(h + 1) * NF],
                        lbs_sb[c][:, col:col + TPT],
                        bd[:],
                        start=True, stop=True,
                    )
                dst = stag[:, q * QUAD * NF:(q + 1) * QUAD * NF]
                if (s * 2 + q) % 2 == 0:
                    nc.vector.tensor_copy(dst, op[:])
                else:
                    nc.scalar.copy(dst, op[:])
            seng = nc.sync if s % 2 == 0 else nc.scalar
            seng.dma_start(
                out_d.ap()[:, s * SBATCH * NF:(s + 1) * SBATCH * NF], stag[:]
            )

    nc.compile()
    _built[key] = nc
    return nc


def make_in_maps(inputs, hostbd=False):
    bf16 = ml_dtypes.bfloat16
    g_full = np.concatenate(
        [inputs["shape_code"], inputs["structure_code"], inputs["pose_code"]],
        axis=-1,
    ).astype(np.float32)  # (8, 128)
    # wt[z, k*64+f] = W[k, f, z]
    wt = np.ascontiguousarray(
        inputs["W"].astype(np.float32).transpose(2, 0, 1).reshape(P, KF)
    ).astype(bf16)
    # biasbd: block-diagonal bias, k' padded to 32
    bias = inputs["bias"].astype(np.float32)
    biasbd = np.zeros((JG, KP, NF), dtype=np.float32)
    for j in range(JG):
        biasbd[j, :K, j * F:(j + 1) * F] = bias

    lbs = inputs["lbs_weights"].astype(np.float32)
    in_maps = []
    for b in range(B):
        # lbs4[j*32+k', tau*128+i] = lbs[b, (tau*128+i)*4+j, k']
        lb = lbs[b].reshape(NCOLS, JG, K).transpose(1, 2, 0)  # (JG, K, 8192)
        lbs4 = np.zeros((JG, KP, NCOLS), dtype=np.float32)
        lbs4[:, :K, :] = lb
        m = {"lbs": np.ascontiguousarray(lbs4.reshape(P, NCOLS)).astype(bf16)}
        if hostbd:
            # bd = blockdiag(local^T + bias^T), local = einsum('kfz,z->kf')
            local = np.einsum(
                "kfz,z->kf", inputs["W"].astype(np.float32), g_full[b]
            ) + bias
            bdh = np.zeros((JG, KP, NF), dtype=np.float32)
            for j in range(JG):
                bdh[j, :K, j * F:(j + 1) * F] = local
            m["bd"] = bdh.reshape(P, NF).astype(bf16)
        else:
            m["grep"] = np.ascontiguousarray(
                np.broadcast_to(g_full[b][:, None], (P, P))
            ).astype(bf16)
            m["wt"] = wt
            m["biasbd"] = biasbd.reshape(P, NF).astype(bf16)
        in_maps.append(m)
    return in_maps


LAST_RESULT = None


def kernel(**inputs) -> np.ndarray:
    global LAST_RESULT
    hostbd = os.environ.get("LFE_HOSTBD", "0") == "1"
    nc = _build(hostbd)
    in_maps = make_in_maps(inputs, hostbd)
    res = bass_utils.run_bass_kernel_spmd(
        nc,
        in_maps,
        core_ids=list(range(B)),
        trace=os.environ.get("LFE_TRACE", "0") == "1",
    )
    LAST_RESULT = res
    outs = []
    for b in range(B):
        o = np.asarray(res.results[b]["out"]).astype(np.float32)
        # out_d[p, tau*256 + j*64 + f] = out[(tau*128+p)*4+j, f]
        o = o.reshape(P, NTILES, JG, F).transpose(1, 0, 2, 3).reshape(T, F)
        outs.append(o)
    return np.stack(outs, axis=0)


if __name__ == "__main__":
    rng = np.random.default_rng(0)
    inputs = {
        "shape_code": rng.standard_normal((B, 64), dtype=np.float32),
        "structure_code": rng.standard_normal((B, 32), dtype=np.float32),
        "pose_code": rng.standard_normal((B, 32), dtype=np.float32),
        "lbs_weights": rng.random((B, T, K), dtype=np.float32),
        "W": rng.standard_normal((K, F, Z), dtype=np.float32),
        "bias": rng.standard_normal((K, F), dtype=np.float32),
    }
    out = kernel(**inputs)
    g = np.concatenate(
        [inputs["shape_code"], inputs["structure_code"], inputs["pose_code"]], -1
    )
    local = np.einsum("kfz,bz->bkf", inputs["W"], g) + inputs["bias"][None]
    ref = np.einsum("btk,bkf->btf", inputs["lbs_weights"], local)
    err = np.abs(out - ref).max() / np.abs(ref).max()
    print("rel err:", err)


# revision 10
# speedup vs baseline: 2.1564x; 1.1298x over previous
"""Trainium2 Bass kernel for nn_LocalFeatureEncoder.

Computes, for B=8 batches on 8 NeuronCores (batch b -> core b):
    g      = concat(shape_code, structure_code, pose_code)      # (B, 128)
    local  = einsum('kfz,bz->bkf', W, g) + bias                 # (B, 24, 64)
    out    = einsum('btk,bkf->btf', lbs_weights, local)         # (B, 32768, 64)

Design (v2 — memory-roofline oriented):
  * Host pre-shuffles lbs into bf16 [128, 8192]: partition (j,k') holds
    lbs[t=(tau*128+i)*4+j, k'] at column tau*128+i (k' zero-padded 24->32).
    This puts the contraction axis on partitions, so the main loop needs
    NO PE transposes and no transpose copies.
  * local is computed on device: flat[i, k*64+f] = sum_z g[z] W[k,f,z] via
    3 matmuls (lhsT = g replicated), then scattered into a block-diagonal
    bf16 rhs bd[128, 256] with 4 tiny SBUF->SBUF DMAs + bias add.
  * Main loop: 64 matmuls [128,128]^T @ [128,256] -> PSUM f32, drained in
    [128,512] pairs by alternating DVE/ACT cast-copies to bf16 staging,
    stored with 8 x 512KB DMAs.
  * bf16 end-to-end (inputs host-cast, output host-upcast): per-core HBM
    traffic ~6.8 MB vs 11.5 MB for f32.
  * Input lbs chunks stream on the sync HWDGE ring; stage-1 constants ride
    the scalar ring so they are not FIFO-blocked behind the bulk load.
"""

import os

import numpy as np
import ml_dtypes

import concourse.bass as bass
import concourse.bacc as bacc
import concourse.tile as tile
from concourse import mybir
from concourse import bass_utils
from contextlib import ExitStack

B, T, K, Z, F = 8, 32768, 24, 128, 64
P = 128
JG = 4                  # t-offsets per partition block
KP = 32                 # padded joints per block (24 used)
TPT = 128               # t-groups (columns of stationary) per tile
NTILES = T // (JG * TPT)       # 64
NF = JG * F                    # 256 bd/matmul output cols
NCOLS = NTILES * TPT           # 8192 lbs cols
IN_CHUNKS = 4
CHUNK_COLS = NCOLS // IN_CHUNKS    # 2048
TILES_PER_CHUNK = NTILES // IN_CHUNKS  # 16
PAIR = 2                # matmul tiles per PSUM bank drain
SBATCH = 8              # tiles per output store (8*256 cols bf16 = 4KB/part)
NSTORES = NTILES // SBATCH     # 8
KF = K * F              # 1536

_built = {}


def _build(hostbd=False):
    key = ("hostbd" if hostbd else "nc")
    if key in _built:
        return _built[key]

    f32 = mybir.dt.float32
    bf16 = mybir.dt.bfloat16
    nc = bacc.Bacc("TRN2", target_bir_lowering=False, debug=False)

    lbs_d = nc.dram_tensor("lbs", (P, NCOLS), bf16, kind="ExternalInput")
    if hostbd:
        bd_d = nc.dram_tensor("bd", (P, NF), bf16, kind="ExternalInput")
    else:
        grep_d = nc.dram_tensor("grep", (P, P), bf16, kind="ExternalInput")
        wt_d = nc.dram_tensor("wt", (P, KF), bf16, kind="ExternalInput")
        biasbd_d = nc.dram_tensor("biasbd", (P, NF), bf16, kind="ExternalInput")
    out_d = nc.dram_tensor("out", (P, NTILES * NF), bf16, kind="ExternalOutput")

    with tile.TileContext(nc) as tc, ExitStack() as ctx:
        # chunk schedule in tiles: small first chunks let the loop start early
        chunk_tiles = [8, 8, 16, 16, 16]
        const = ctx.enter_context(tc.tile_pool(name="const", bufs=1))
        lbs_pool = ctx.enter_context(
            tc.tile_pool(name="lbs_pool", bufs=len(chunk_tiles))
        )
        if not hostbd:
            psS = ctx.enter_context(
                tc.tile_pool(name="psS", bufs=2, space=bass.MemorySpace.PSUM)
            )
        psO = ctx.enter_context(
            tc.tile_pool(
                name="psO", bufs=(4 if hostbd else 3), space=bass.MemorySpace.PSUM
            )
        )
        stag_pool = ctx.enter_context(tc.tile_pool(name="stag_pool", bufs=3))

        # ---- stage-1 constants FIRST on the sync ring: they are small and
        # complete in ~1.5us; lbs queues behind them FIFO ----
        if hostbd:
            bd = const.tile([P, NF], bf16)
            nc.sync.dma_start(bd[:], bd_d.ap())
        else:
            wt_sb = const.tile([P, KF], bf16)
            nc.sync.dma_start(wt_sb[:], wt_d.ap())
            grep_sb = const.tile([P, P], bf16)
            nc.sync.dma_start(grep_sb[:], grep_d.ap())
            biasbd_sb = const.tile([P, NF], bf16)
            nc.sync.dma_start(biasbd_sb[:], biasbd_d.ap())

        # ---- bulk lbs stream on the sync ring ----
        # tile_of[ti] -> (sbuf tile, col offset within it)
        tile_of = {}
        lbs_sb = []
        t0i = 0
        for nt in chunk_tiles:
            t = lbs_pool.tile([P, nt * TPT], bf16)
            nc.sync.dma_start(
                t[:], lbs_d.ap()[:, t0i * TPT:(t0i + nt) * TPT]
            )
            lbs_sb.append(t)
            for i in range(nt):
                tile_of[t0i + i] = (t, i * TPT)
            t0i += nt

        if not hostbd:
            # ---- stage 1: flat[i, k*64+f] = sum_z g[z] W[k,f,z] (rows identical) ----
            bdt = const.tile([P, NF], bf16)
            nc.vector.memset(bdt[:], 0.0)

            flat_sb = const.tile([P, KF], bf16)
            for n in range(3):
                fp = psS.tile([P, 512], f32, tag="s1")
                nc.tensor.matmul(
                    fp[:], grep_sb[:], wt_sb[:, n * 512:(n + 1) * 512],
                    start=True, stop=True,
                )
                if n == 1:
                    nc.scalar.copy(flat_sb[:, n * 512:(n + 1) * 512], fp[:])
                else:
                    nc.vector.tensor_copy(flat_sb[:, n * 512:(n + 1) * 512], fp[:])

            # scatter row 0 of flat into the diagonal blocks of bdt; the
            # scalar ring (Q10) is empty and ACT is otherwise idle here
            for j in range(JG):
                nc.scalar.dma_start(
                    bdt[j * KP:j * KP + K, j * F:(j + 1) * F], flat_sb[0:1, :]
                )
            bd = const.tile([P, NF], bf16)
            nc.vector.tensor_add(bd[:], bdt[:], biasbd_sb[:])

        # ---- main loop: 64 matmuls, drained in 2-bank quads, 8 store batches ----
        QUAD = 4
        for s in range(NSTORES):
            stag = stag_pool.tile([P, SBATCH * NF], bf16)
            for q in range(SBATCH // QUAD):
                op = psO.tile([P, QUAD * NF], f32)
                for h in range(QUAD):
                    ti = s * SBATCH + q * QUAD + h
                    lt, col = tile_of[ti]
                    nc.tensor.matmul(
                        op[:, h * NF:(h + 1) * NF],
                        lt[:, col:col + TPT],
                        bd[:],
                        start=True, stop=True,
                    )
                dst = stag[:, q * QUAD * NF:(q + 1) * QUAD * NF]
                if (s * 2 + q) % 2 == 0:
                    nc.vector.tensor_copy(dst, op[:])
                else:
                    nc.scalar.copy(dst, op[:])
            # stores ride the scalar ring ONLY: the sync ring carries the
            # input stream and a store queued behind it would stall stag reuse
            nc.scalar.dma_start(
                out_d.ap()[:, s * SBATCH * NF:(s + 1) * SBATCH * NF], stag[:]
            )

    nc.compile()
    _built[key] = nc
    return nc


def make_in_maps(inputs, hostbd=False):
    bf16 = ml_dtypes.bfloat16
    g_full = np.concatenate(
        [inputs["shape_code"], inputs["structure_code"], inputs["pose_code"]],
        axis=-1,
    ).astype(np.float32)  # (8, 128)
    # wt[z, k*64+f] = W[k, f, z]
    wt = np.ascontiguousarray(
        inputs["W"].astype(np.float32).transpose(2, 0, 1).reshape(P, KF)
    ).astype(bf16)
    # biasbd: block-diagonal bias, k' padded to 32
    bias = inputs["bias"].astype(np.float32)
    biasbd = np.zeros((JG, KP, NF), dtype=np.float32)
    for j in range(JG):
        biasbd[j, :K, j * F:(j + 1) * F] = bias

    lbs = inputs["lbs_weights"].astype(np.float32)
    in_maps = []
    for b in range(B):
        # lbs4[j*32+k', tau*128+i] = lbs[b, (tau*128+i)*4+j, k']
        lb = lbs[b].reshape(NCOLS, JG, K).transpose(1, 2, 0)  # (JG, K, 8192)
        lbs4 = np.zeros((JG, KP, NCOLS), dtype=np.float32)
        lbs4[:, :K, :] = lb
        m = {"lbs": np.ascontiguousarray(lbs4.reshape(P, NCOLS)).astype(bf16)}
        if hostbd:
            # bd = blockdiag(local^T + bias^T), local = einsum('kfz,z->kf')
            local = np.einsum(
                "kfz,z->kf", inputs["W"].astype(np.float32), g_full[b]
            ) + bias
            bdh = np.zeros((JG, KP, NF), dtype=np.float32)
            for j in range(JG):
                bdh[j, :K, j * F:(j + 1) * F] = local
            m["bd"] = bdh.reshape(P, NF).astype(bf16)
        else:
            m["grep"] = np.ascontiguousarray(
                np.broadcast_to(g_full[b][:, None], (P, P))
            ).astype(bf16)
            m["wt"] = wt
            m["biasbd"] = biasbd.reshape(P, NF).astype(bf16)
        in_maps.append(m)
    return in_maps


LAST_RESULT = None


def kernel(**inputs) -> np.ndarray:
    global LAST_RESULT
    hostbd = os.environ.get("LFE_HOSTBD", "0") == "1"
    nc = _build(hostbd)
    in_maps = make_in_maps(inputs, hostbd)
    res = bass_utils.run_bass_kernel_spmd(
        nc,
        in_maps,
        core_ids=list(range(B)),
        trace=os.environ.get("LFE_TRACE", "0") == "1",
    )
    LAST_RESULT = res
    outs = []
    for b in range(B):
        o = np.asarray(res.results[b]["out"]).astype(np.float32)
        # out_d[p, tau*256 + j*64 + f] = out[(tau*128+p)*4+j, f]
        o = o.reshape(P, NTILES, JG, F).transpose(1, 0, 2, 3).reshape(T, F)
        outs.append(o)
    return np.stack(outs, axis=0)


if __name__ == "__main__":
    rng = np.random.default_rng(0)
    inputs = {
        "shape_code": rng.standard_normal((B, 64), dtype=np.float32),
        "structure_code": rng.standard_normal((B, 32), dtype=np.float32),
        "pose_code": rng.standard_normal((B, 32), dtype=np.float32),
        "lbs_weights": rng.random((B, T, K), dtype=np.float32),
        "W": rng.standard_normal((K, F, Z), dtype=np.float32),
        "bias": rng.standard_normal((K, F), dtype=np.float32),
    }
    out = kernel(**inputs)
    g = np.concatenate(
        [inputs["shape_code"], inputs["structure_code"], inputs["pose_code"]], -1
    )
    local = np.einsum("kfz,bz->bkf", inputs["W"], g) + inputs["bias"][None]
    ref = np.einsum("btk,bkf->btf", inputs["lbs_weights"], local)
    err = np.abs(out - ref).max() / np.abs(ref).max()
    print("rel err:", err)


# revision 12
# speedup vs baseline: 2.2048x; 1.0225x over previous
"""Trainium2 Bass kernel for nn_LocalFeatureEncoder.

Computes, for B=8 batches on 8 NeuronCores (batch b -> core b):
    g      = concat(shape_code, structure_code, pose_code)      # (B, 128)
    local  = einsum('kfz,bz->bkf', W, g) + bias                 # (B, 24, 64)
    out    = einsum('btk,bkf->btf', lbs_weights, local)         # (B, 32768, 64)

Design (v2 — memory-roofline oriented):
  * Host pre-shuffles lbs into bf16 [128, 8192]: partition (j,k') holds
    lbs[t=(tau*128+i)*4+j, k'] at column tau*128+i (k' zero-padded 24->32).
    This puts the contraction axis on partitions, so the main loop needs
    NO PE transposes and no transpose copies.
  * local is computed on device: flat[i, k*64+f] = sum_z g[z] W[k,f,z] via
    3 matmuls (lhsT = g replicated), then scattered into a block-diagonal
    bf16 rhs bd[128, 256] with 4 tiny SBUF->SBUF DMAs + bias add.
  * Main loop: 64 matmuls [128,128]^T @ [128,256] -> PSUM f32, drained in
    [128,512] pairs by alternating DVE/ACT cast-copies to bf16 staging,
    stored with 8 x 512KB DMAs.
  * bf16 end-to-end (inputs host-cast, output host-upcast): per-core HBM
    traffic ~6.8 MB vs 11.5 MB for f32.
  * Input lbs chunks stream on the sync HWDGE ring; stage-1 constants ride
    the scalar ring so they are not FIFO-blocked behind the bulk load.
"""

import os

import numpy as np
import ml_dtypes

import concourse.bass as bass
import concourse.bacc as bacc
import concourse.tile as tile
from concourse import mybir
from concourse import bass_utils
from contextlib import ExitStack

B, T, K, Z, F = 8, 32768, 24, 128, 64
P = 128
JG = 4                  # t-offsets per partition block
KP = 32                 # padded joints per block (24 used)
TPT = 128               # t-groups (columns of stationary) per tile
NTILES = T // (JG * TPT)       # 64
NF = JG * F                    # 256 bd/matmul output cols
NCOLS = NTILES * TPT           # 8192 lbs cols
IN_CHUNKS = 4
CHUNK_COLS = NCOLS // IN_CHUNKS    # 2048
TILES_PER_CHUNK = NTILES // IN_CHUNKS  # 16
PAIR = 2                # matmul tiles per PSUM bank drain
SBATCH = 8              # tiles per output store (8*256 cols bf16 = 4KB/part)
NSTORES = NTILES // SBATCH     # 8
KF = K * F              # 1536

_built = {}


def _build(hostbd=False):
    key = ("hostbd" if hostbd else "nc")
    if key in _built:
        return _built[key]

    f32 = mybir.dt.float32
    bf16 = mybir.dt.bfloat16
    nc = bacc.Bacc("TRN2", target_bir_lowering=False, debug=False)

    lbs_d = nc.dram_tensor("lbs", (P, NCOLS), bf16, kind="ExternalInput")
    if hostbd:
        bd_d = nc.dram_tensor("bd", (P, NF), bf16, kind="ExternalInput")
    else:
        grep_d = nc.dram_tensor("grep", (P, P), bf16, kind="ExternalInput")
        wt_d = nc.dram_tensor("wt", (P, KF), bf16, kind="ExternalInput")
        biasbd_d = nc.dram_tensor("biasbd", (P, NF), bf16, kind="ExternalInput")
    out_d = nc.dram_tensor("out", (P, NTILES * NF), bf16, kind="ExternalOutput")

    with tile.TileContext(nc) as tc, ExitStack() as ctx:
        # chunk schedule in tiles: small first chunks let the loop start early
        chunk_tiles = [4, 4, 8, 16, 16, 16]
        const = ctx.enter_context(tc.tile_pool(name="const", bufs=1))
        lbs_pool = ctx.enter_context(
            tc.tile_pool(name="lbs_pool", bufs=len(chunk_tiles))
        )
        if not hostbd:
            psS = ctx.enter_context(
                tc.tile_pool(name="psS", bufs=2, space=bass.MemorySpace.PSUM)
            )
        psW = ctx.enter_context(
            tc.tile_pool(name="psW", bufs=1, space=bass.MemorySpace.PSUM)
        )
        psO = ctx.enter_context(
            tc.tile_pool(name="psO", bufs=3, space=bass.MemorySpace.PSUM)
        )
        stag_pool = ctx.enter_context(tc.tile_pool(name="stag_pool", bufs=3))

        # ---- PE pre-warm: ~8 x 512-col dummy matmuls keep the PE busy for
        # ~3.4us from t~6.7us so the HAM un-throttles the clock (1.2->2.4GHz)
        # right as the first lbs chunk lands ----
        wz = const.tile([P, 512], bf16)
        nc.vector.memset(wz[:], 0.0)
        pw = psW.tile([P, 512], f32)
        NWARM = 8
        for i in range(NWARM):
            nc.tensor.matmul(
                pw[:], wz[:, 0:P], wz[:],
                start=(i == 0), stop=(i == NWARM - 1),
            )

        # ---- stage-1 constants FIRST on the sync ring: they are small and
        # complete in ~1.5us; lbs queues behind them FIFO ----
        if hostbd:
            bd = const.tile([P, NF], bf16)
            nc.sync.dma_start(bd[:], bd_d.ap())
        else:
            wt_sb = const.tile([P, KF], bf16)
            nc.sync.dma_start(wt_sb[:], wt_d.ap())
            grep_sb = const.tile([P, P], bf16)
            nc.sync.dma_start(grep_sb[:], grep_d.ap())
            biasbd_sb = const.tile([P, NF], bf16)
            nc.sync.dma_start(biasbd_sb[:], biasbd_d.ap())

        # ---- bulk lbs stream on the sync ring ----
        # tile_of[ti] -> (sbuf tile, col offset within it)
        tile_of = {}
        lbs_sb = []
        t0i = 0
        for nt in chunk_tiles:
            t = lbs_pool.tile([P, nt * TPT], bf16)
            nc.sync.dma_start(
                t[:], lbs_d.ap()[:, t0i * TPT:(t0i + nt) * TPT]
            )
            lbs_sb.append(t)
            for i in range(nt):
                tile_of[t0i + i] = (t, i * TPT)
            t0i += nt

        if not hostbd:
            # ---- stage 1: flat[i, k*64+f] = sum_z g[z] W[k,f,z] (rows identical) ----
            bdt = const.tile([P, NF], bf16)
            nc.vector.memset(bdt[:], 0.0)

            flat_sb = const.tile([P, KF], bf16)
            for n in range(3):
                fp = psS.tile([P, 512], f32, tag="s1")
                nc.tensor.matmul(
                    fp[:], grep_sb[:], wt_sb[:, n * 512:(n + 1) * 512],
                    start=True, stop=True,
                )
                if n == 1:
                    nc.scalar.copy(flat_sb[:, n * 512:(n + 1) * 512], fp[:])
                else:
                    nc.vector.tensor_copy(flat_sb[:, n * 512:(n + 1) * 512], fp[:])

            # scatter row 0 of flat into the diagonal blocks of bdt; the
            # scalar ring (Q10) is empty and ACT is otherwise idle here
            for j in range(JG):
                nc.scalar.dma_start(
                    bdt[j * KP:j * KP + K, j * F:(j + 1) * F], flat_sb[0:1, :]
                )
            bd = const.tile([P, NF], bf16)
            nc.vector.tensor_add(bd[:], bdt[:], biasbd_sb[:])

        # ---- main loop: 64 matmuls, drained in 2-bank quads, 8 store batches ----
        QUAD = 4
        for s in range(NSTORES):
            stag = stag_pool.tile([P, SBATCH * NF], bf16)
            for q in range(SBATCH // QUAD):
                op = psO.tile([P, QUAD * NF], f32)
                for h in range(QUAD):
                    ti = s * SBATCH + q * QUAD + h
                    lt, col = tile_of[ti]
                    nc.tensor.matmul(
                        op[:, h * NF:(h + 1) * NF],
                        lt[:, col:col + TPT],
                        bd[:],
                        start=True, stop=True,
                    )
                dst = stag[:, q * QUAD * NF:(q + 1) * QUAD * NF]
                if (s * 2 + q) % 2 == 0:
                    nc.vector.tensor_copy(dst, op[:])
                else:
                    nc.scalar.copy(dst, op[:])
            # early stores ride the scalar ring ONLY (the sync ring still
            # carries the input stream; a store FIFO'd behind it would stall
            # stag reuse); once the input has drained, alternate rings so the
            # final stores overlap across both queues
            seng = nc.sync if (s >= 5 and s % 2 == 1) else nc.scalar
            seng.dma_start(
                out_d.ap()[:, s * SBATCH * NF:(s + 1) * SBATCH * NF], stag[:]
            )

    nc.compile()
    _built[key] = nc
    return nc


def make_in_maps(inputs, hostbd=False):
    bf16 = ml_dtypes.bfloat16
    g_full = np.concatenate(
        [inputs["shape_code"], inputs["structure_code"], inputs["pose_code"]],
        axis=-1,
    ).astype(np.float32)  # (8, 128)
    # wt[z, k*64+f] = W[k, f, z]
    wt = np.ascontiguousarray(
        inputs["W"].astype(np.float32).transpose(2, 0, 1).reshape(P, KF)
    ).astype(bf16)
    # biasbd: block-diagonal bias, k' padded to 32
    bias = inputs["bias"].astype(np.float32)
    biasbd = np.zeros((JG, KP, NF), dtype=np.float32)
    for j in range(JG):
        biasbd[j, :K, j * F:(j + 1) * F] = bias

    lbs = inputs["lbs_weights"].astype(np.float32)
    in_maps = []
    for b in range(B):
        # lbs4[j*32+k', tau*128+i] = lbs[b, (tau*128+i)*4+j, k']
        lb = lbs[b].reshape(NCOLS, JG, K).transpose(1, 2, 0)  # (JG, K, 8192)
        lbs4 = np.zeros((JG, KP, NCOLS), dtype=np.float32)
        lbs4[:, :K, :] = lb
        m = {"lbs": np.ascontiguousarray(lbs4.reshape(P, NCOLS)).astype(bf16)}
        if hostbd:
            # bd = blockdiag(local^T + bias^T), local = einsum('kfz,z->kf')
            local = np.einsum(
                "kfz,z->kf", inputs["W"].astype(np.float32), g_full[b]
            ) + bias
            bdh = np.zeros((JG, KP, NF), dtype=np.float32)
            for j in range(JG):
                bdh[j, :K, j * F:(j + 1) * F] = local
            m["bd"] = bdh.reshape(P, NF).astype(bf16)
        else:
            m["grep"] = np.ascontiguousarray(
                np.broadcast_to(g_full[b][:, None], (P, P))
            ).astype(bf16)
            m["wt"] = wt
            m["biasbd"] = biasbd.reshape(P, NF).astype(bf16)
        in_maps.append(m)
    return in_maps


LAST_RESULT = None


def kernel(**inputs) -> np.ndarray:
    global LAST_RESULT
    hostbd = os.environ.get("LFE_HOSTBD", "0") == "1"
    nc = _build(hostbd)
    in_maps = make_in_maps(inputs, hostbd)
    res = bass_utils.run_bass_kernel_spmd(
        nc,
        in_maps,
        core_ids=list(range(B)),
        trace=os.environ.get("LFE_TRACE", "0") == "1",
    )
    LAST_RESULT = res
    outs = []
    for b in range(B):
        o = np.asarray(res.results[b]["out"]).astype(np.float32)
        # out_d[p, tau*256 + j*64 + f] = out[(tau*128+p)*4+j, f]
        o = o.reshape(P, NTILES, JG, F).transpose(1, 0, 2, 3).reshape(T, F)
        outs.append(o)
    return np.stack(outs, axis=0)


if __name__ == "__main__":
    rng = np.random.default_rng(0)
    inputs = {
        "shape_code": rng.standard_normal((B, 64), dtype=np.float32),
        "structure_code": rng.standard_normal((B, 32), dtype=np.float32),
        "pose_code": rng.standard_normal((B, 32), dtype=np.float32),
        "lbs_weights": rng.random((B, T, K), dtype=np.float32),
        "W": rng.standard_normal((K, F, Z), dtype=np.float32),
        "bias": rng.standard_normal((K, F), dtype=np.float32),
    }
    out = kernel(**inputs)
    g = np.concatenate(
        [inputs["shape_code"], inputs["structure_code"], inputs["pose_code"]], -1
    )
    local = np.einsum("kfz,bz->bkf", inputs["W"], g) + inputs["bias"][None]
    ref = np.einsum("btk,bkf->btf", inputs["lbs_weights"], local)
    err = np.abs(out - ref).max() / np.abs(ref).max()
    print("rel err:", err)
